# revision 1
# baseline (speedup 1.0000x reference)
"""Trainium2 Bass kernel for ConstrainedMLP (B=262144, 12->256->256->12 MLP + constraints).

Data-parallel across 8 NeuronCores: batch is split 8 x 32768, tiny weights
replicated. Per core:
  - load x in natural [128, c, 12] batch-major tiles
  - PE-transpose 128x12 chunks -> feature-major xT [12, 512]
  - L1/L2/L3 as float32r matmuls (1 cycle/column at N=512) with weights stationary
  - relu+bias fused into the PSUM->SBUF copy on the scalar engine
  - PE-transpose y back to batch-major, run the constraint epilogue on
    vector/scalar/gpsimd engines, DMA out
"""
import sys

sys.path.insert(0, "/opt/trn_rl_repo")

from contextlib import ExitStack

import numpy as np

import concourse.bass as bass
import concourse.tile as tile
from concourse import bacc, mybir
from concourse.bass_utils import run_bass_kernel_spmd
from concourse.masks import make_identity

B, IN, HID, OUT = 262144, 12, 256, 12
NCORES = 8
BC = B // NCORES          # 32768 batch rows per core
SUP = 512                 # batch rows per supertile
NCH = SUP // 128          # 4 chunks of 128 rows
NSUP = BC // SUP          # 64 supertiles
F32 = mybir.dt.float32
F32R = mybir.dt.float32r
ALU = mybir.AluOpType
ACTF = mybir.ActivationFunctionType
AX = mybir.AxisListType


def _build(nsup=NSUP, debug_raw_y=False):
    bc = nsup * SUP
    nc = bacc.Bacc(None)
    inp_h = nc.declare_dram_parameter("inp", [bc, IN], F32, isOutput=False)
    W1_h = nc.declare_dram_parameter("W1", [IN, HID], F32, isOutput=False)
    b1_h = nc.declare_dram_parameter("b1", [HID], F32, isOutput=False)
    W2_h = nc.declare_dram_parameter("W2", [HID, HID], F32, isOutput=False)
    b2_h = nc.declare_dram_parameter("b2", [HID], F32, isOutput=False)
    W3_h = nc.declare_dram_parameter("W3", [HID, OUT], F32, isOutput=False)
    b3_h = nc.declare_dram_parameter("b3", [OUT], F32, isOutput=False)
    out_h = nc.declare_dram_parameter("out", [bc, IN], F32, isOutput=True)

    with tile.TileContext(nc) as tc, ExitStack() as ctx:
        const = ctx.enter_context(tc.tile_pool(name="const", bufs=1))
        xpool = ctx.enter_context(tc.tile_pool(name="xp", bufs=6))
        spool = ctx.enter_context(tc.tile_pool(name="sp", bufs=4))
        opool = ctx.enter_context(tc.tile_pool(name="op", bufs=6))
        pps = ctx.enter_context(tc.tile_pool(name="pps", bufs=1, space="PSUM"))
        pps2 = ctx.enter_context(tc.tile_pool(name="pps2", bufs=1, space="PSUM"))

        # ---- constants (loaded once) ----
        W1sb = const.tile([IN, HID], F32)
        nc.sync.dma_start(out=W1sb[:], in_=W1_h[:])
        W2sb = const.tile([128, 2, HID], F32)
        nc.sync.dma_start(out=W2sb[:], in_=W2_h[:].rearrange("(kt p) j -> p kt j", p=128))
        W3sb = const.tile([128, 2, OUT], F32)
        nc.sync.dma_start(out=W3sb[:], in_=W3_h[:].rearrange("(kt p) j -> p kt j", p=128))

        b1sb = const.tile([128, 2], F32)
        nc.sync.dma_start(out=b1sb[:], in_=b1_h[:].rearrange("(m p) -> p m", p=128))
        b2sb = const.tile([128, 2], F32)
        nc.sync.dma_start(out=b2sb[:], in_=b2_h[:].rearrange("(m p) -> p m", p=128))
        b3sb = const.tile([OUT, 1], F32)
        nc.sync.dma_start(out=b3sb[:], in_=b3_h[:].unsqueeze(1))

        id128f = const.tile([128, 128], F32)
        make_identity(nc, id128f[:])
        id128r = id128f[:]
        id12 = const.tile([OUT, OUT], F32)
        make_identity(nc, id12[:])

        inp_r = inp_h[:].rearrange("(s c p) f -> s p c f", p=128, c=NCH)
        out_r = out_h[:].rearrange("(s c p) f -> s p c f", p=128, c=NCH)

        for s in range(nsup):
            # ---- load x ----
            x = xpool.tile([128, NCH, IN], F32, tag="x")
            nc.sync.dma_start(out=x[:], in_=inp_r[s])
            xr = xpool.tile([128, NCH, IN], F32, tag="xr")
            nc.vector.tensor_copy(xr[:], x[:])

            # ---- transpose x chunks: [128,12] -> [12,128] ----
            xT_ps = pps.tile([IN, NCH, 128], F32, tag="xT")
            for c in range(NCH):
                nc.tensor.transpose(xT_ps[:, c], xr[:, c], id128r)
            xT = spool.tile([IN, NCH * 128], F32, tag="xTs")
            nc.vector.tensor_copy(xT[:], xT_ps[:].rearrange("f c p -> f (c p)"))

            # ---- L1: h1T[m*128+j, n] ----
            h1_ps = [pps.tile([128, SUP], F32, tag=f"h1_{m}", name=f"h1ps{m}")
                     for m in range(2)]
            for m in range(2):
                nc.tensor.matmul(h1_ps[m][:], W1sb[:, m * 128:(m + 1) * 128], xT[:],
                                 start=True, stop=True)
            h1T = spool.tile([128, 2, SUP], F32, tag="h1T")
            for m in range(2):
                nc.scalar.activation(h1T[:, m], h1_ps[m][:], ACTF.Relu,
                                     bias=b1sb[:, m:m + 1], scale=1.0)

            # ---- L2 ----
            h2_ps = [pps.tile([128, SUP], F32, tag=f"h2_{m}", name=f"h2ps{m}")
                     for m in range(2)]
            for m in range(2):
                nc.tensor.matmul(h2_ps[m][:], W2sb[:, 0, m * 128:(m + 1) * 128],
                                 h1T[:, 0], start=True, stop=False)
                nc.tensor.matmul(h2_ps[m][:], W2sb[:, 1, m * 128:(m + 1) * 128],
                                 h1T[:, 1], start=False, stop=True)
            h2T = spool.tile([128, 2, SUP], F32, tag="h2T")
            for m in range(2):
                nc.scalar.activation(h2T[:, m], h2_ps[m][:], ACTF.Relu,
                                     bias=b2sb[:, m:m + 1], scale=1.0)

            # ---- L3: yT [12, SUP] ----
            yT_ps = pps2.tile([OUT, SUP], F32, tag="yT")
            nc.tensor.matmul(yT_ps[:], W3sb[:, 0], h2T[:, 0], start=True, stop=False)
            nc.tensor.matmul(yT_ps[:], W3sb[:, 1], h2T[:, 1], start=False, stop=True)
            yTb = spool.tile([OUT, SUP], F32, tag="yTb")
            nc.scalar.activation(yTb[:], yT_ps[:], ACTF.Identity,
                                 bias=b3sb[:, 0:1], scale=1.0)

            # ---- transpose back: y [128, c, 12] ----
            y_ps = pps2.tile([128, NCH, OUT], F32, tag="y", bufs=2)
            for c in range(NCH):
                nc.tensor.transpose(y_ps[:, c], yTb[:, c * 128:(c + 1) * 128], id12[:])

            # ================= epilogue (batch-major) =================
            if debug_raw_y:
                oy = opool.tile([128, NCH, IN], F32, tag="oy")
                nc.vector.tensor_copy(oy[:], y_ps[:])
                nc.sync.dma_start(out=out_r[s], in_=oy[:])
                continue
            o = opool.tile([128, NCH, IN], F32, tag="o")
            # tanh of everything (col 6 fixed below)
            nc.scalar.activation(o[:], y_ps[:], ACTF.Tanh)
            # pts = max(sigmoid(y6), prev)
            sg = opool.tile([128, NCH, 1], F32, tag="sg")
            nc.scalar.activation(sg[:], y_ps[:, :, 6:7], ACTF.Sigmoid)
            nc.vector.tensor_tensor(o[:, :, 6:7], sg[:], x[:, :, 6:7], ALU.max)

            # --- pos: clip to unit sphere (scale = min(1/dist, 1)) ---
            t3 = opool.tile([128, NCH, 3], F32, tag="t3")
            nc.vector.tensor_tensor(t3[:], o[:, :, 0:3], o[:, :, 0:3], ALU.mult)
            r1 = opool.tile([128, NCH], F32, tag="r1")
            nc.vector.tensor_reduce(r1[:], t3[:], AX.X, ALU.add)
            nc.scalar.activation(r1[:], r1[:], ACTF.Sqrt)        # dist
            nc.vector.reciprocal(r1[:], r1[:])                   # 1/dist
            nc.vector.tensor_scalar(r1[:], r1[:], 1.0, None, ALU.min)
            nc.vector.tensor_tensor(
                o[:, :, 0:3], o[:, :, 0:3],
                r1[:].unsqueeze(2).broadcast_to([128, NCH, 3]), ALU.mult)

            # --- clus ---
            dp = opool.tile([128, NCH, 3], F32, tag="dp")
            nc.gpsimd.tensor_tensor(dp[:], x[:, :, 0:3], x[:, :, 7:10], ALU.subtract)
            d3 = opool.tile([128, NCH, 3], F32, tag="d3")
            nc.gpsimd.tensor_tensor(d3[:], o[:, :, 7:10], x[:, :, 7:10], ALU.subtract)
            # dd = <delta, deputy>
            t3b = opool.tile([128, NCH, 3], F32, tag="t3b")
            nc.gpsimd.tensor_tensor(t3b[:], d3[:], dp[:], ALU.mult)
            dd = opool.tile([128, NCH], F32, tag="dd")
            nc.vector.tensor_reduce(dd[:], t3b[:], AX.X, ALU.add)
            # dnorm
            nc.vector.tensor_tensor(t3b[:], dp[:], dp[:], ALU.mult)
            dn = opool.tile([128, NCH], F32, tag="dn")
            nc.vector.tensor_reduce(dn[:], t3b[:], AX.X, ALU.add)
            nc.scalar.activation(dn[:], dn[:], ACTF.Sqrt)
            nc.vector.reciprocal(dn[:], dn[:])                   # 1/|deputy|
            # w = (dd > 0) * (1/|deputy|)   -> offset = clus - w * deputy
            msk = opool.tile([128, NCH], F32, tag="msk")
            nc.vector.tensor_single_scalar(msk[:], dd[:], 0.0, ALU.is_gt)
            nc.vector.tensor_tensor(dn[:], dn[:], msk[:], ALU.mult)
            off = opool.tile([128, NCH, 3], F32, tag="off")
            nc.vector.tensor_tensor(
                off[:], dp[:], dn[:].unsqueeze(2).broadcast_to([128, NCH, 3]), ALU.mult)
            nc.vector.tensor_tensor(off[:], o[:, :, 7:10], off[:], ALU.subtract)
            # cdist, select
            nc.gpsimd.tensor_tensor(t3b[:], off[:], off[:], ALU.mult)
            cd = opool.tile([128, NCH], F32, tag="cd")
            nc.vector.tensor_reduce(cd[:], t3b[:], AX.X, ALU.add)
            nc.scalar.activation(cd[:], cd[:], ACTF.Sqrt)
            cm = opool.tile([128, NCH], F32, tag="cm")
            nc.gpsimd.tensor_single_scalar(cm[:], cd[:], 1.0, ALU.is_gt)
            nc.vector.reciprocal(cd[:], cd[:])
            nc.vector.tensor_tensor(
                off[:], off[:], cd[:].unsqueeze(2).broadcast_to([128, NCH, 3]), ALU.mult)
            # blend: clus + (cdist>1) * (off/cdist - clus)
            nc.vector.tensor_tensor(off[:], off[:], o[:, :, 7:10], ALU.subtract)
            nc.vector.tensor_tensor(
                off[:], off[:], cm[:].unsqueeze(2).broadcast_to([128, NCH, 3]), ALU.mult)
            nc.vector.tensor_tensor(o[:, :, 7:10], o[:, :, 7:10], off[:], ALU.add)

            # --- sun: project to unit circle ---
            t2 = opool.tile([128, NCH, 2], F32, tag="t2")
            nc.gpsimd.tensor_tensor(t2[:], o[:, :, 10:12], o[:, :, 10:12], ALU.mult)
            sn = opool.tile([128, NCH], F32, tag="sn")
            nc.vector.tensor_reduce(sn[:], t2[:], AX.X, ALU.add)
            nc.scalar.activation(sn[:], sn[:], ACTF.Sqrt)
            nc.vector.reciprocal(sn[:], sn[:])
            nc.vector.tensor_tensor(
                o[:, :, 10:12], o[:, :, 10:12],
                sn[:].unsqueeze(2).broadcast_to([128, NCH, 2]), ALU.mult)

            # ---- store ----
            nc.sync.dma_start(out=out_r[s], in_=o[:])

    nc.finalize()
    return nc


_CACHED_NC = None


def kernel(**inputs: np.ndarray) -> np.ndarray:
    global _CACHED_NC
    if _CACHED_NC is None:
        _CACHED_NC = _build()
    nc = _CACHED_NC
    inp = np.ascontiguousarray(inputs["inp"], dtype=np.float32)
    shared = {k: np.ascontiguousarray(inputs[k], dtype=np.float32)
              for k in ("W1", "b1", "W2", "b2", "W3", "b3")}
    in_maps = [dict(shared, inp=inp[i * BC:(i + 1) * BC]) for i in range(NCORES)]
    res = run_bass_kernel_spmd(nc, in_maps, list(range(NCORES)))
    return np.concatenate([res.results[i]["out"] for i in range(NCORES)], axis=0)

